# revision 27
# baseline (speedup 1.0000x reference)
"""DendriticFullyConnected Trainium2 kernel — mixed bf16 / fp8-DoubleRow.

Math (per reference):
  x_c  = x[:, :409];  x_nc = x[:, 409:]
  state = sigmoid(x_nc @ W_non.T + b_non) - 1
  cluster = (x_c * coeff) @ W_nmda.T          # coeff = [1,2,...,2,1]
  pre = cluster + state
  out = pre^2 / (0.25 + pre^2)

Strategy: data-parallel over batch on 8 cores (1024 rows each), weights
replicated.  The contraction splits by precision sensitivity:

  nmda part (K=409->512, 4 k-tiles)  : bf16.  cluster hits the Hill directly
    (sigma~2, gain ~1), so fp8 here costs ~5e-2 rel err.  bf16 keeps it at
    ~3e-3 and runs at 1 cycle/row (216 ns per [128k,128o]x[128k,512b] MM).
  non part (K=3687+bias->3840, 15 pairs of k-tiles): fp8 e4m3 with
    perf_mode=DoubleRow (2 fp8 weights per PE cell -> 256-deep contraction
    per 216 ns matmul = 2x bf16 FLOPs; the DR LDWEIGHTS (135 ns) hides
    behind the previous matmul).  The sigmoid's <=0.25 gain squashes the
    fp8 quantization noise (measured 6.4e-3 rel-l2 end to end vs the 2e-2
    gate).  W_non/b_non are pre-scaled by 64 so sigma~1 lands mid e4m3
    range (away from subnormals); 1/64 is folded into the sigmoid's scale.

DMA: a single HWDGE queue processes ~1 packet / 10-15 ns regardless of
size, so throughput is set by the per-partition contiguous chunk: ~70 GB/s
at 1 KB packets vs ~390 GB/s at 4 KB.  All device layouts are therefore
partition-major with multi-KB contiguous per-partition rows, and the whole
input fill is 9 large DMAs instead of ~90 (each trigger also costs ~0.65 us
on the issuing engine):
  xnm [128p, kt(4),  b(1024)] bf16  (2/6 KB rows; 3 chunk DMAs on scalar)
  xnn [128p, kp(15), i(2), b(1024)] f8  (30 KB rows; 1 DMA on scalar)
  wnm [128p, ot(32), kt(4), o(128)] bf16  (4-20 KB rows; 3 chunks on sync)
  wnn [128p, ot(32), kp(15), i(2), o(128)] f8  (3.84 KB rows; 1 DMA/ot on
    sync, double-buffered, logical k = kp*256 + i*128 + p)
Outputs store per o-tile as one [128, 1024] bf16 DMA (2 KB rows) on the
sync queue; gpsimd SWDGE (~40 GB/s, 70 ns/packet software descriptor gen)
is not used at all.  The fill is HBM-bound (~380-430 GB/s aggregate,
~half per queue while both are busy), so each queue's issue order is
strict need-order; the first matmul (~13 us: ~7 us fixed framework
preamble + trigger + kick latency + wmA/xm(kt0) transfers on separate
queues) starts as soon as its operands can physically arrive, and the PE
never starves afterwards.  Run-to-run exec varies ~+/-10 us with device
DVFS state (matmul durations 379 vs 434/454 ns across runs); comparisons
need min-of-N.

Device: outT[o, b] = sum_k wt[k, o] xt[k, b] with W-stationary matmuls
(lhsT = w tile, rhs = cached x), two PSUM groups (nmda / non) per o-tile,
then the sigmoid + Hill epilogue on ACT/DVE — sigmoid(ACT, psum-read,
scale folds the fp8 prescale), pre=nm-sig (DVE), pre^2 (ACT), den=sq+KD,
rec=1/den, out=1-KD*rec (DVE).  Output bf16; host upcasts.

Phase A runs the bf16 nmda phases of the first OT_AHEAD o-tiles (o-tiles
0-3 k-OUTER so each arriving xm k-chunk unlocks 8 matmuls while the fill
streams; psum-budget-capped at 4 o-tiles), phases B/C are the plain
o-outer loop (non sweep + epilogue, then nmda+non+epilogue for the rest).
Sync engine program order per iteration: prefetch wn(ot+4) (depth 4: the
wn stream only gets HBM leftovers during the fill), then the store
trigger for ot (which blocks the engine on the epilogue's ob sem,
harmlessly).  The last o-tile accumulates (bh, column-half)-outer into
column slices of two psum banks so after the very last matmul only one
256-col epilogue chain and a 64 KB store remain.
"""

import numpy as np
import ml_dtypes

B = 8192
IN_F = 4096
OUT_F = 4096
IC = 409                      # clustering synapses
INC = IN_F - IC               # 3687
KD = 0.25                     # Hill k_d = k_a^n = 0.5^2
NCORES = 8
BLOC = B // NCORES            # 1024
OT = OUT_F // 128             # 32 output-row tiles
NBH = BLOC // 512             # 2 batch halves (512 = max matmul free dim)
OT_AHEAD = 7                  # o-tiles whose nmda phase covers the x fill

KNM_PAD = 512                 # nmda contraction, padded (4 k-tiles, bf16)
KNM_TILES = 4
KNN = INC + 1                 # 3688: non contraction + bias row
KP = 15                       # fp8 DoubleRow k-pairs (15 * 256 = 3840)
KNN_PAD = KP * 256
S_W = 64.0                    # fp8 pre-scale on W_non/b_non

_nc_cache = []


def _build():
    import concourse.bacc as bacc
    import concourse.tile as tile
    import concourse.mybir as mybir

    f32 = mybir.dt.float32
    bf16 = mybir.dt.bfloat16
    f8 = mybir.dt.float8e4
    ACT = mybir.ActivationFunctionType
    DR = mybir.MatmulPerfMode.DoubleRow

    nc = bacc.Bacc(None, target_bir_lowering=False)
    xnm = nc.dram_tensor("xnm", [128, KNM_TILES * BLOC], bf16, kind="ExternalInput")
    xnn = nc.dram_tensor("xnn", [128, KP * 2 * BLOC], f8, kind="ExternalInput")
    wnm = nc.dram_tensor("wnm", [128, OT * KNM_TILES * 128], bf16, kind="ExternalInput")
    wnn = nc.dram_tensor("wnn", [128, OT * KP * 2 * 128], f8, kind="ExternalInput")
    outT = nc.dram_tensor("outT", [OUT_F, BLOC], bf16, kind="ExternalOutput")

    # wm chunk split (o-tile ranges): first covers the k-outer group, the
    # second covers phase A's per-ot sweeps, the rest is lazy (phase C).
    WM_CHUNKS = [(0, 4), (4, 9), (9, OT)]
    # xm chunk split (kt ranges): kt0 gates the very first matmuls.
    XM_CHUNKS = [(0, 1), (1, 2), (2, KNM_TILES)]
    # xn chunk split (kp ranges), one per HWDGE queue: the fill is
    # HBM-bound (~380-430 GB/s aggregate; ~190/queue when both are busy),
    # so the split halves xn's completion and the tail kps (needed ~4 us
    # into phase B's first sweep) ride the more-loaded scalar queue.
    XN_CHUNKS = [(0, 10), (10, KP)]

    with tile.TileContext(nc) as tc:
        with (
            # Pool ORDER is load-bearing: pools stack in allocation order, so
            # the tiles the PE streams from during phase A (wmA, xm) sit at
            # the bottom, the epilogue pools (nm/tmp/ob, ~72 KB) form a
            # buffer zone, and the big fill targets that are DMA-written
            # while phase A runs (xn, wmB/C, wn) land far away.  DMA writes
            # into the same SBUF region as the PE's rhs stream were measured
            # to stretch matmuls ~1.6x (566-634 ns vs 379).  (Explicit
            # side="left/right" split regressed ALL matmuls to 454 ns —
            # right-side allocation itself is slow — so single-side only.)
            tc.tile_pool(name="wmapool", bufs=1) as wmapool,
            tc.tile_pool(name="xmapool", bufs=1) as xmapool,
            tc.tile_pool(name="nmpool", bufs=24) as nmpool,
            tc.tile_pool(name="tmp", bufs=8) as tmp,
            tc.tile_pool(name="opool", bufs=4) as opool,
            tc.tile_pool(name="xmbcpool", bufs=1) as xmbcpool,
            tc.tile_pool(name="xnpool", bufs=1) as xnpool,
            tc.tile_pool(name="wmbcpool", bufs=1) as wmbcpool,
            tc.tile_pool(name="wnpool", bufs=5) as wnpool,
            tc.tile_pool(name="psum", bufs=8, space="PSUM") as psum,
        ):
            def osl(ot):
                return slice(ot * 128, (ot + 1) * 128)

            def bsl(bh):
                return slice(bh * 512, (bh + 1) * 512)

            # ── consolidated input fill (see module docstring) ─────────
            # Strict need-order per queue under the shared HBM cap (the
            # scalar HWDGE queue starts ~1.5 us later than sync: its first
            # trigger trails the ACT_TABLE_LOAD).  The first matmul needs
            # wmA+xm(kt0); fetching them on DIFFERENT queues concurrently
            # beats serializing on sync:
            #   sync:   wmA(ot0-3), xm(kt1), xn(kp0-9), wn1, wn2, wn3,
            #           then the wn/store loop
            #   scalar: xm(kt0), xm(kt2-3), wmB(ot4-8), wn0, xn(kp10-14),
            #           wmC(ot9-31)
            def wm_chunk(pool, lo, hi):
                t = pool.tile([128, hi - lo, KNM_TILES, 128], bf16,
                              tag=f"wm{lo}", name=f"wm_{lo}")
                src = wnm[:, lo * KNM_TILES * 128 : hi * KNM_TILES * 128].rearrange(
                    "p (ot kt o) -> p ot kt o", ot=hi - lo, kt=KNM_TILES
                )
                return t, src

            def xm_chunk(pool, lo, hi):
                t = pool.tile([128, hi - lo, BLOC], bf16, tag=f"xm{lo}",
                              name=f"xm_{lo}")
                src = xnm[:, lo * BLOC : hi * BLOC].rearrange(
                    "p (kt b) -> p kt b", kt=hi - lo
                )
                return t, src

            wmA, wmA_src = wm_chunk(wmapool, *WM_CHUNKS[0])
            wmB, wmB_src = wm_chunk(wmbcpool, *WM_CHUNKS[1])
            wmC, wmC_src = wm_chunk(wmbcpool, *WM_CHUNKS[2])
            wm_tiles = [
                (WM_CHUNKS[0][0], WM_CHUNKS[0][1], wmA),
                (WM_CHUNKS[1][0], WM_CHUNKS[1][1], wmB),
                (WM_CHUNKS[2][0], WM_CHUNKS[2][1], wmC),
            ]
            xm_tiles = []
            xm_views = []
            for i, (lo, hi) in enumerate(XM_CHUNKS):
                t, src = xm_chunk(xmapool if i == 0 else xmbcpool, lo, hi)
                xm_tiles.append((lo, hi, t))
                xm_views.append((t, src))
            xn = xnpool.tile([128, KP, 2, BLOC], f8, tag="xn")

            def xn_view(lo, hi):
                return (
                    xn[:, lo:hi],
                    xnn[:, lo * 2 * BLOC : hi * 2 * BLOC].rearrange(
                        "p (kp i b) -> p kp i b", kp=hi - lo, i=2
                    ),
                )

            wn_tiles = {}

            def prefetch_wn(ot, engine=None):
                if ot not in wn_tiles and ot < OT:
                    wg = wnpool.tile([128, KP, 2, 128], f8, tag="wn", name=f"wn_{ot}")
                    (engine or nc.sync).dma_start(
                        wg[:],
                        wnn[:, ot * KP * 256 : (ot + 1) * KP * 256].rearrange(
                            "p (kp i o) -> p kp i o", kp=KP, i=2
                        ),
                    )
                    wn_tiles[ot] = wg

            def get_wn(ot):
                prefetch_wn(ot)
                return wn_tiles.pop(ot)

            nc.sync.dma_start(wmA[:], wmA_src)
            nc.scalar.dma_start(xm_views[0][0][:], xm_views[0][1])
            nc.sync.dma_start(xm_views[1][0][:], xm_views[1][1])
            nc.scalar.dma_start(xm_views[2][0][:], xm_views[2][1])
            nc.sync.dma_start(wmB[:], wmB_src)
            prefetch_wn(0, nc.scalar)
            xn1_dst, xn1_src = xn_view(*XN_CHUNKS[0])
            nc.sync.dma_start(xn1_dst, xn1_src)
            xn2_dst, xn2_src = xn_view(*XN_CHUNKS[1])
            nc.scalar.dma_start(xn2_dst, xn2_src)
            prefetch_wn(1)
            prefetch_wn(2)
            prefetch_wn(3)
            nc.scalar.dma_start(wmC[:], wmC_src)

            def wm_lhsT(ot, kt):
                for lo, hi, t in wm_tiles:
                    if lo <= ot < hi:
                        return t[:, ot - lo, kt, :]
                raise AssertionError(ot)

            def xm_rhs(kt, bh):
                for lo, hi, t in xm_tiles:
                    if lo <= kt < hi:
                        return t[:, kt - lo, bsl(bh)]
                raise AssertionError(kt)

            def nmda_group(ots):
                # k-OUTER over a group of o-tiles (<=4: psum budget): during
                # the x fill each arriving xm k-chunk unlocks len(ots)*2
                # matmuls, keeping the PE fed while xnm streams in.
                psn = [
                    [
                        psum.tile([128, 512], f32, tag="ps", name=f"psn_{ot}_{i}")
                        for i in range(NBH)
                    ]
                    for ot in ots
                ]
                for kt in range(KNM_TILES):
                    for j, ot in enumerate(ots):
                        for bh in range(NBH):
                            nc.tensor.matmul(
                                psn[j][bh][:],
                                lhsT=wm_lhsT(ot, kt),
                                rhs=xm_rhs(kt, bh),
                                start=(kt == 0),
                                stop=(kt == KNM_TILES - 1),
                            )
                nms = []
                for j, ot in enumerate(ots):
                    nm = []
                    for bh in range(NBH):
                        t = nmpool.tile([128, 512], f32, tag="nm", name=f"nm_{ot}_{bh}")
                        nc.scalar.copy(t[:], psn[j][bh][:])
                        nm.append(t)
                    nms.append(nm)
                return nms

            def nmda_phase(ot):
                return nmda_group([ot])[0]

            def non_phase(ot):
                wg = get_wn(ot)
                ps = [
                    psum.tile([128, 512], f32, tag="ps", name=f"ps_{ot}_{i}")
                    for i in range(NBH)
                ]
                for kp in range(KP):
                    for bh in range(NBH):
                        nc.tensor.matmul(
                            ps[bh][:],
                            lhsT=wg[:, kp, :, :],
                            rhs=xn[:, kp, :, bsl(bh)],
                            start=(kp == 0),
                            stop=(kp == KP - 1),
                            perf_mode=DR,
                        )
                return ps

            def last_ot(ot, nm_pair):
                # Final o-tile: (bh, column-half)-outer accumulation into
                # four [128, 256] psum tiles, so after the very last matmul
                # only one 256-col epilogue chain + 64 KB store remain
                # (everything earlier hides under the later matmul groups).
                wg = get_wn(ot)
                psf = [
                    psum.tile([128, 512], f32, tag="ps", name=f"psl_{bh}")
                    for bh in range(NBH)
                ]
                # column-half views: same psum banks/tag, half-range
                # accumulation groups (start resets only the addressed range)
                ps = [
                    [psf[bh][:, h * 256 : (h + 1) * 256] for h in range(2)]
                    for bh in range(NBH)
                ]
                sig = [
                    [
                        tmp.tile([128, 256], f32, tag="th", name=f"sigh_{bh}_{h}")
                        for h in range(2)
                    ]
                    for bh in range(NBH)
                ]
                rec = [
                    [
                        tmp.tile([128, 256], f32, tag="th", name=f"rech_{bh}_{h}")
                        for h in range(2)
                    ]
                    for bh in range(NBH)
                ]
                ob = opool.tile([128, BLOC], bf16, tag="o", name=f"ob_{ot}")

                def hsl(bh, h):
                    return slice(bh * 512 + h * 256, bh * 512 + (h + 1) * 256)

                def chain(bh, h):
                    nm_s = slice(h * 256, (h + 1) * 256)
                    yield lambda: nc.scalar.activation(
                        sig[bh][h][:], ps[bh][h][:], ACT.Sigmoid,
                        scale=-1.0 / S_W,
                    )
                    yield lambda: nc.vector.tensor_sub(
                        sig[bh][h][:], nm_pair[bh][:, nm_s], sig[bh][h][:]
                    )
                    yield lambda: nc.scalar.activation(
                        nm_pair[bh][:, nm_s], sig[bh][h][:], ACT.Square
                    )
                    yield lambda: nc.vector.tensor_scalar_add(
                        sig[bh][h][:], nm_pair[bh][:, nm_s], KD
                    )
                    yield lambda: nc.vector.reciprocal_approx_fast(
                        rec[bh][h][:], sig[bh][h][:]
                    )
                    yield lambda: nc.vector.tensor_scalar(
                        ob[:, hsl(bh, h)], rec[bh][h][:], -KD, 1.0,
                        mybir.AluOpType.mult, mybir.AluOpType.add,
                    )

                for bh in range(NBH):
                    for h in range(2):
                        for kp in range(KP):
                            nc.tensor.matmul(
                                ps[bh][h][:],
                                lhsT=wg[:, kp, :, :],
                                rhs=xn[:, kp, :, hsl(bh, h)],
                                start=(kp == 0),
                                stop=(kp == KP - 1),
                                perf_mode=DR,
                            )
                for bh in range(NBH):
                    chains = [chain(bh, h) for h in range(2)]
                    for _op in range(6):
                        for h in range(2):
                            next(chains[h])()
                    for h in range(2):
                        nc.sync.dma_start(
                            outT[osl(ot), hsl(bh, h)], ob[:, hsl(bh, h)]
                        )

            def epilogue_pair(ot, ps_pair, nm_pair):
                # psum = S_W*(z+b); pre = nm - sigmoid(-(z+b));
                # out = pre^2/(KD+pre^2) = 1 - KD/(KD+pre^2).  Op-major
                # order so ACT and DVE overlap across the batch halves.
                sig = [
                    tmp.tile([128, 512], f32, tag="t", name=f"sig_{ot}_{bh}")
                    for bh in range(NBH)
                ]
                rec = [
                    tmp.tile([128, 512], f32, tag="t", name=f"rec_{ot}_{bh}")
                    for bh in range(NBH)
                ]
                ob = opool.tile([128, BLOC], bf16, tag="o", name=f"ob_{ot}")

                def chain(bh):
                    yield lambda: nc.scalar.activation(
                        sig[bh][:], ps_pair[bh][:], ACT.Sigmoid,
                        scale=-1.0 / S_W,
                    )
                    yield lambda: nc.vector.tensor_sub(
                        sig[bh][:], nm_pair[bh][:], sig[bh][:]
                    )  # := pre
                    yield lambda: nc.scalar.activation(
                        nm_pair[bh][:], sig[bh][:], ACT.Square
                    )
                    yield lambda: nc.vector.tensor_scalar_add(
                        sig[bh][:], nm_pair[bh][:], KD
                    )
                    yield lambda: nc.vector.reciprocal_approx_fast(
                        rec[bh][:], sig[bh][:]
                    )
                    yield lambda: nc.vector.tensor_scalar(
                        ob[:, bsl(bh)], rec[bh][:], -KD, 1.0,
                        mybir.AluOpType.mult, mybir.AluOpType.add,
                    )

                chains = [chain(bh) for bh in range(NBH)]
                for _op in range(6):
                    for bh in range(NBH):
                        next(chains[bh])()
                nc.sync.dma_start(outT[osl(ot), :], ob[:])

            # ── Phase A: nmda for the first OT_AHEAD o-tiles (xnm only).
            # wn0-3 already prefetched above (depth 4: during the fill the
            # wn stream only gets HBM leftovers, ~115 GB/s = 4.3 us/tile vs
            # 6.5 us/tile consumption; a 2-deep prefetch stalled mid-run). ──
            nm_ahead = []
            nm_ahead.extend(nmda_group([0, 1, 2, 3]))
            for ot in range(4, OT_AHEAD):
                nm_ahead.append(nmda_phase(ot))

            # ── Phase B: non + epilogue for the ahead o-tiles ──
            for ot in range(OT_AHEAD):
                prefetch_wn(ot + 4)
                ps = non_phase(ot)
                epilogue_pair(ot, ps, nm_ahead[ot])

            # ── Phase C: remaining o-tiles, plain o-outer loop ──
            for ot in range(OT_AHEAD, OT - 1):
                prefetch_wn(ot + 4)
                nm = nmda_phase(ot)
                ps = non_phase(ot)
                epilogue_pair(ot, ps, nm)
            nm = nmda_phase(OT - 1)
            last_ot(OT - 1, nm)
    nc.compile()
    return nc


def _warmup():
    """Tiny throwaway NEFF run: the first execution after session start
    occasionally dies with NRT_EXEC_UNIT_UNRECOVERABLE; absorb that here."""
    import concourse.bacc as bacc
    import concourse.tile as tile
    import concourse.mybir as mybir
    from concourse.bass_utils import run_bass_kernel_spmd

    nc = bacc.Bacc(None, target_bir_lowering=False)
    a = nc.dram_tensor("a", [128, 128], mybir.dt.float32, kind="ExternalInput")
    b = nc.dram_tensor("b", [128, 128], mybir.dt.float32, kind="ExternalOutput")
    with tile.TileContext(nc) as tc:
        with tc.tile_pool(name="p", bufs=1) as pool:
            t = pool.tile([128, 128], mybir.dt.float32)
            nc.sync.dma_start(t[:], a[:])
            nc.sync.dma_start(b[:], t[:])
    nc.compile()
    ins = [{"a": np.zeros((128, 128), np.float32)} for _ in range(NCORES)]
    for _ in range(3):
        try:
            run_bass_kernel_spmd(nc, ins, core_ids=list(range(NCORES)))
            return
        except Exception:
            continue


def kernel(x, W_nmda, W_non, b_non):
    from concourse.bass_utils import run_bass_kernel_spmd

    x = np.asarray(x, dtype=np.float32)
    W_nmda = np.asarray(W_nmda, dtype=np.float32)
    W_non = np.asarray(W_non, dtype=np.float32)
    b_non = np.asarray(b_non, dtype=np.float32)

    coeff = np.full((IC,), 2.0, dtype=np.float32)
    coeff[0] = 1.0
    coeff[-1] = 1.0

    bf16 = ml_dtypes.bfloat16
    f8 = ml_dtypes.float8_e4m3

    # x, nmda part: [128 p, kt, B] bf16 (k = kt*128 + p)
    xTm = np.zeros((KNM_PAD, B), dtype=np.float32)
    xTm[0:IC] = x[:, :IC].T
    xTm = (
        xTm.reshape(KNM_TILES, 128, B).transpose(1, 0, 2)
    ).astype(bf16)  # [128, kt, B]

    # x, non part: [128 p, kp, i, B] fp8 (k = kp*256 + i*128 + p)
    xTn = np.zeros((KNN_PAD, B), dtype=np.float32)
    xTn[0:INC] = x[:, IC:].T
    xTn[INC] = 1.0  # bias row
    xTn = (
        xTn.reshape(KP, 2, 128, B).transpose(2, 0, 1, 3)
    ).astype(f8)  # [128, kp, i, B]

    # W, nmda part: [128 p, ot, kt, o] bf16
    wTm = np.zeros((KNM_PAD, OUT_F), dtype=np.float32)
    wTm[0:IC] = (W_nmda * coeff[None, :]).T
    wnm = np.ascontiguousarray(
        wTm.reshape(KNM_TILES, 128, OT, 128).transpose(1, 2, 0, 3)
    ).reshape(128, OT * KNM_TILES * 128).astype(bf16)

    # W, non part (pre-scaled by S_W): [128 p, ot, kp, i, o] fp8
    wTn = np.zeros((KNN_PAD, OUT_F), dtype=np.float32)
    wTn[0:INC] = W_non.T * S_W
    wTn[INC] = b_non * S_W
    wnn = np.ascontiguousarray(
        wTn.reshape(KP, 2, 128, OT, 128).transpose(2, 3, 0, 1, 4)
    ).reshape(128, OT * KP * 256).astype(f8)

    in_maps = [
        {
            "xnm": np.ascontiguousarray(
                xTm[:, :, c * BLOC : (c + 1) * BLOC]
            ).reshape(128, KNM_TILES * BLOC),
            "xnn": np.ascontiguousarray(
                xTn[:, :, :, c * BLOC : (c + 1) * BLOC]
            ).reshape(128, KP * 2 * BLOC),
            "wnm": wnm,
            "wnn": wnn,
        }
        for c in range(NCORES)
    ]

    if not _nc_cache:
        _warmup()
        _nc_cache.append(_build())
    nc = _nc_cache[0]

    res = None
    last_exc = None
    for _attempt in range(3):
        try:
            res = run_bass_kernel_spmd(nc, in_maps, core_ids=list(range(NCORES)))
            break
        except Exception as e:  # transient device errors (e.g. first-run NRT hiccup)
            last_exc = e
    if res is None:
        raise last_exc

    global LAST_RESULT
    LAST_RESULT = res

    out = np.empty((B, OUT_F), dtype=np.float32)
    for c in range(NCORES):
        out[c * BLOC : (c + 1) * BLOC] = res.results[c]["outT"].astype(np.float32).T
    return out


LAST_RESULT = None
